# revision 13
# baseline (speedup 1.0000x reference)
"""DeepGraphSAGE (4-layer, 128-dim) Trainium2 Bass kernel, 8-way SPMD.

v2: the per-edge gather is done with batched `dma_gather` (one SWDGE
descriptor-generation per 8-chunk run — 1024 rows, the SWDGE ring capacity —
instead of one indirect DMA per 128-edge chunk, which was the v1 bottleneck:
Pool engine 95% busy on desc-gen).  The node
feature table is replicated in DRAM as bf16 and addressed as 25000 pair-rows
of 512B (dma_gather indices are signed int16, so single rows of a 50000-row
table would overflow; pairs halve the index range and keep descriptors at the
512B bandwidth crossover).  Each 128-edge chunk does two bf16 one-hot matmuls
(even/odd pair member, with parity pre-masked into the host-built dst tables)
accumulating the segment-sum in PSUM.  Dense SAGE matmuls run in bf16 with
BatchNorm folded into the weights; biases/accumulation stay fp32.  The
AllGather that republishes node features each layer runs in bf16 and is
split into 4 segment collectives (node ids are interleaved so each segment's
AllGather output is contiguous), issued as soon as the corresponding dense
groups finish so most of the collective hides under the gather stream.
"""

import os
import sys
from dataclasses import dataclass

import numpy as np

for _p in ("/opt/trn_rl_repo", "/root/.axon_site/_ro/trn_rl_repo"):
    if os.path.isdir(_p) and _p not in sys.path:
        sys.path.append(_p)

import ml_dtypes

import concourse.bass as bass
import concourse.bacc as bacc
import concourse.mybir as mybir
import concourse.tile as tile
from concourse.masks import make_identity

F32 = mybir.dt.float32
BF16 = mybir.dt.bfloat16
I16 = mybir.dt.int16
RELU = mybir.ActivationFunctionType.Relu
BF = ml_dtypes.bfloat16
EPS = 1e-5
CH = 128          # edges per chunk (PE contraction dim)
WIN = 64          # segment-sum window width (nodes per PSUM window)
NSEG = 4          # AllGather segments per layer


@dataclass
class Cfg:
    N: int = 50000
    E: int = 800000
    D: int = 128
    C: int = 8                      # cores
    batch_chunks: int = 8           # chunks per dma_gather run (1024 descs =
                                    # the SWDGE carveout ring; >1024 wedges HW)

    @property
    def NLOC(self):
        return self.N // self.C

    @property
    def NWIN(self):
        return -(-self.NLOC // WIN)

    @property
    def seg_bounds(self):
        """Local-node boundaries of the NSEG AllGather segments."""
        step = -(-self.NLOC // NSEG)
        return [min(s * step, self.NLOC) for s in range(NSEG)] + [self.NLOC]


def _balance_nodes(cfg: Cfg, deg: np.ndarray):
    """Degree-balanced node relabeling: LPT-deal nodes into (core, window)
    buckets so every window's edge count packs 128-edge chunks tightly
    (removes the SPMD max-over-cores padding).  Returns pi (old->new)."""
    import heapq
    NWIN, NLOC = cfg.NWIN, cfg.NLOC
    nbuck = cfg.C * NWIN
    cap = np.array([min(WIN, NLOC - (b % NWIN) * WIN) for b in range(nbuck)],
                   np.int64)
    order = np.argsort(-deg, kind="stable")
    heap = [(0.0, b) for b in range(nbuck)]
    heapq.heapify(heap)
    cnt = np.zeros(nbuck, np.int64)
    esum = np.zeros(nbuck, np.int64)
    pi = np.empty(cfg.N, np.int64)
    base = np.array([(b // NWIN) * NLOC + (b % NWIN) * WIN
                     for b in range(nbuck)], np.int64)
    for v in order:
        while True:
            s, b = heapq.heappop(heap)
            if cnt[b] < cap[b]:
                break
        pi[v] = base[b] + cnt[b]
        cnt[b] += 1
        esum[b] += deg[v]
        if cnt[b] < cap[b]:
            heapq.heappush(heap, (float(esum[b]), b))
    return pi


def _gid_map(cfg: Cfg):
    """local id (c*NLOC+n) -> interleaved global id for the gather table.
    Segment s of the table holds rows [8*L[s], 8*L[s+1]) = concat over cores
    of each core's local slice [L[s], L[s+1]) — i.e. one AllGather output."""
    L = cfg.seg_bounds
    gid = np.empty(cfg.N, np.int64)
    for c in range(cfg.C):
        for s in range(NSEG):
            lo, hi = L[s], L[s + 1]
            ln = hi - lo
            base = cfg.C * lo + c * ln
            gid[c * cfg.NLOC + lo: c * cfg.NLOC + hi] = base + np.arange(ln)
    return gid


def preprocess(cfg: Cfg, edge_index: np.ndarray):
    """Edge schedule: per-core gather-index / dst tables, identical shape
    across cores."""
    src_raw = edge_index[0].astype(np.int64)
    dst_raw = edge_index[1].astype(np.int64)
    deg_raw = np.bincount(dst_raw, minlength=cfg.N)
    pi = _balance_nodes(cfg, deg_raw)
    sigma = np.argsort(pi)               # new -> old
    gid = _gid_map(cfg)
    src_all = pi[src_raw]
    dst_all = pi[dst_raw]
    deg = np.bincount(dst_all, minlength=cfg.N).astype(np.float32)
    inv_deg = (1.0 / np.maximum(deg, 1.0)).astype(np.float32)

    per_core = []
    core_of = dst_all // cfg.NLOC
    for c in range(cfg.C):
        m = core_of == c
        s = src_all[m]
        d = dst_all[m] - c * cfg.NLOC
        order = np.argsort(d, kind="stable")
        per_core.append((s[order], d[order]))

    # per-window chunk counts (max over cores -> SPMD-identical schedule)
    bounds = []
    for c in range(cfg.C):
        d = per_core[c][1]
        bounds.append(np.searchsorted(d, np.arange(0, cfg.NWIN + 1) * WIN))
    nch_w = []
    for w in range(cfg.NWIN):
        mx = 1
        for c in range(cfg.C):
            cnt = bounds[c][w + 1] - bounds[c][w]
            mx = max(mx, -(-int(cnt) // CH))
        nch_w.append(mx)
    T = sum(nch_w)
    slot0 = np.concatenate([[0], np.cumsum(nch_w)])

    # per-chunk tables: pair index (int16) and parity-masked dst offsets
    pair_idx = np.zeros((cfg.C, CH, T), np.int16)
    dst_eo = np.full((cfg.C, CH, 2 * T), -1.0, np.float32)
    for c in range(cfg.C):
        s, d = per_core[c]
        g = gid[s]
        pidx_all = (g >> 1).astype(np.int16)
        par_all = (g & 1).astype(np.int64)
        for w in range(cfg.NWIN):
            lo, hi = bounds[c][w], bounds[c][w + 1]
            n = hi - lo
            nch = -(-int(n) // CH) if n else 0
            t0 = int(slot0[w])
            if not nch:
                continue
            pad = nch * CH - n
            ep = np.concatenate([pidx_all[lo:hi],
                                 np.zeros(pad, np.int16)])
            ed = np.concatenate([d[lo:hi] - w * WIN, np.full(pad, -1.0)])
            epar = np.concatenate([par_all[lo:hi], np.zeros(pad, np.int64)])
            epc = ep.reshape(nch, CH).T              # [CH, nch]
            edc = ed.reshape(nch, CH).T
            eoc = epar.reshape(nch, CH).T
            pair_idx[c, :, t0:t0 + nch] = epc
            dpad = np.where((eoc == 0) & (edc >= 0), edc, -1.0)
            dodd = np.where((eoc == 1) & (edc >= 0), edc, -1.0)
            dst_eo[c, :, 2 * t0:2 * (t0 + nch):2] = dpad
            dst_eo[c, :, 2 * t0 + 1:2 * (t0 + nch) + 1:2] = dodd

    # gather batches: consecutive whole windows, <= batch_chunks chunks each
    batches = []  # (w_start, w_end, t_start, t_end)
    w = 0
    while w < cfg.NWIN:
        w0, t0 = w, int(slot0[w])
        n = 0
        while w < cfg.NWIN and (n + nch_w[w] <= cfg.batch_chunks or w == w0):
            n += nch_w[w]
            w += 1
        batches.append((w0, w, t0, int(slot0[w])))

    # int16 index stream: gathers run over fixed groups of batch_chunks
    # slots.  Per run, num_idxs=(t1-t0)*128 laid out [16, num_idxs/16]
    # (idx i at [i%16, i//16]), replicated to 8 groups of 16 partitions,
    # runs concatenated along the free dim (8 cols/chunk).
    RC = cfg.batch_chunks
    idx16 = np.zeros((cfg.C, 128, 8 * T), np.int16)
    for c in range(cfg.C):
        for t0 in range(0, T, RC):
            t1 = min(t0 + RC, T)
            nb = t1 - t0
            flat = pair_idx[c, :, t0:t1].T.reshape(-1)   # edge i = chunk*128+p
            blk = flat.reshape(nb * 8, 16).T              # [16, nb*8]
            idx16[c, :16, 8 * t0:8 * t1] = blk
        for grp in range(1, 8):
            idx16[c, grp * 16:(grp + 1) * 16, :] = idx16[c, :16, :]

    maxn = max(nch_w)
    iota2 = np.tile(np.arange(WIN, dtype=np.float32), (CH, 2 * maxn))
    return dict(inv_deg=inv_deg, idx16=idx16, dst_eo=dst_eo,
                nch_w=nch_w, batches=batches, T=T, maxn=maxn, iota2=iota2,
                pi=pi, sigma=sigma)


def fold_weights(Wp, bp, Wl, bl, Wr, bn_gamma, bn_beta, bn_mean, bn_var):
    """Fold BN (eval mode) into the SAGE weights: relu(bn(h)) ==
    relu(agg @ Wl' + x @ Wr' + c)."""
    a = bn_gamma / np.sqrt(bn_var + EPS)           # [4, D]
    b = bn_beta - bn_mean * a                      # [4, D]
    Wl_f = (Wl * a[:, None, :]).astype(np.float32)
    Wr_f = (Wr * a[:, None, :]).astype(np.float32)
    c_f = (bl * a + b).astype(np.float32)          # [4, D]
    return Wl_f, Wr_f, c_f


def build_program(cfg: Cfg, nch_w, batches, T, maxn, ablate=frozenset()):
    NLOC, NWIN, D, C = cfg.NLOC, cfg.NWIN, cfg.D, cfg.C
    n_dense = -(-NLOC // 512)
    n_tp = -(-NLOC // 128)
    NB = max(cfg.batch_chunks, maxn)
    slot0 = np.concatenate([[0], np.cumsum(nch_w)])
    L = cfg.seg_bounds
    # dense group after which segment s can be AllGathered
    seg_trigger = [-(-L[s + 1] // 512) - 1 for s in range(NSEG)]
    rg = [list(range(C))]

    nc = bacc.Bacc("TRN2", target_bir_lowering=False, debug=False,
                   num_devices=C, num_swdge_queues=4)

    x_in = nc.dram_tensor("x_slice", [NLOC, D], BF16, kind="ExternalInput")
    idx_in = nc.dram_tensor("idx16", [128, 8 * T], I16, kind="ExternalInput")
    dsteo_in = nc.dram_tensor("dst_eo", [CH, 2 * T], BF16,
                              kind="ExternalInput")
    iota_in = nc.dram_tensor("iota2", [CH, 2 * maxn * WIN], BF16,
                             kind="ExternalInput")
    invdeg_in = nc.dram_tensor("inv_deg_sl", [1, NLOC], F32,
                               kind="ExternalInput")
    w_in = nc.dram_tensor("weights", [9, D, D], BF16, kind="ExternalInput")
    b_in = nc.dram_tensor("biases", [9, D], F32, kind="ExternalInput")
    y_out = nc.dram_tensor("y", [NLOC, D], F32, kind="ExternalOutput")

    if "bare" in ablate:
        with tile.TileContext(nc) as tc:
            with tc.tile_pool(name="sb", bufs=2) as sb:
                t = sb.tile([128, D], BF16)
                nc.sync.dma_start(out=t[:], in_=x_in[:128, :])
                t2 = sb.tile([128, D], F32)
                nc.vector.tensor_copy(out=t2[:], in_=t[:])
                nc.sync.dma_start(out=y_out[:128, :], in_=t2[:])
        nc.compile()
        return nc

    with tile.TileContext(nc) as tc:
        with (
            tc.tile_pool(name="consts", bufs=1) as cs,
            tc.tile_pool(name="work", bufs=2) as wk,
            tc.tile_pool(name="big", bufs=1) as bg,
            tc.tile_pool(name="psum", bufs=2, space="PSUM") as ps,
            tc.tile_pool(name="dram", bufs=1, space="DRAM") as dr,
        ):
            # ---------- persistent loads ----------
            idx_sb = cs.tile([128, 8 * T], I16)
            dsteo_sb = cs.tile([CH, 2 * T], BF16)
            iota_sb = cs.tile([CH, 2 * maxn * WIN], BF16)
            invdeg_sl = cs.tile([1, NLOC], F32)
            w_sb = cs.tile([128, 9 * D], BF16)
            b_sb = cs.tile([128, 9], F32)
            ident = cs.tile([128, 128], BF16)
            make_identity(nc, ident[:])
            nc.sync.dma_start(out=idx_sb[:], in_=idx_in[:])
            nc.sync.dma_start(out=dsteo_sb[:], in_=dsteo_in[:])
            nc.sync.dma_start(out=iota_sb[:], in_=iota_in[:])
            nc.sync.dma_start(out=invdeg_sl[:], in_=invdeg_in[:])
            nc.sync.dma_start(
                out=w_sb[:].rearrange("p (k d) -> p k d", d=D),
                in_=w_in[:].rearrange("k p d -> p k d"),
            )
            nc.sync.dma_start(out=b_sb[:], in_=b_in[:].rearrange("k p -> p k"))

            def W(k):
                return w_sb[:, k * D:(k + 1) * D]

            def B(k):
                return b_sb[:, k:k + 1]

            # inv_deg broadcast across partitions via K=1 outer product
            ones_sb = cs.tile([1, 128], F32)
            nc.vector.memset(ones_sb[:], 1.0)
            invdeg_bc = cs.tile([128, NLOC], F32)
            for gidx in range(n_dense):
                lo, hi = gidx * 512, min((gidx + 1) * 512, NLOC)
                ps_bc = ps.tile([128, 512], F32, space="PSUM", tag="dense")
                nc.tensor.matmul(out=ps_bc[:, :hi - lo], lhsT=ones_sb[:],
                                 rhs=invdeg_sl[:, lo:hi], start=True, stop=True)
                nc.vector.tensor_copy(out=invdeg_bc[:, lo:hi],
                                      in_=ps_bc[:, :hi - lo])

            stage = bg.tile([128, n_tp * 128], BF16)   # row-major staging

            def transpose_in(dst_T, src_dram):
                """DRAM rows [NLOC, D] bf16 -> feature-major SBUF [D, NLOC]."""
                for i in range(n_tp):
                    lo, hi = i * 128, min((i + 1) * 128, NLOC)
                    p = hi - lo
                    xr = wk.tile([128, D], BF16, tag="xr")
                    nc.sync.dma_start(out=xr[:p], in_=src_dram[lo:hi, :])
                    ps_t = ps.tile([128, 128], BF16, space="PSUM", tag="tp")
                    nc.tensor.transpose(out=ps_t[:, :p], in_=xr[:p],
                                        identity=ident[:p, :p])
                    nc.vector.tensor_copy(out=dst_T[:, lo:hi], in_=ps_t[:, :p])

            def transpose_tile(src_T, i):
                lo, hi = i * 128, min((i + 1) * 128, NLOC)
                p = hi - lo
                ps_t = ps.tile([128, 128], BF16, space="PSUM", tag="tp")
                nc.tensor.transpose(out=ps_t[:p, :], in_=src_T[:, lo:hi],
                                    identity=ident[:, :])
                nc.vector.tensor_copy(out=stage[:p, i * 128:(i + 1) * 128],
                                      in_=ps_t[:p, :])

            def publish_rows(dst_dram, lo, hi):
                for i in range(lo // 128, -(-hi // 128)):
                    rlo, rhi = i * 128, min((i + 1) * 128, hi)
                    p = rhi - rlo
                    nc.sync.dma_start(out=dst_dram[rlo:rhi, :],
                                      in_=stage[:p, i * 128:(i + 1) * 128])

            # ---------- prologue: load + project ----------
            xT = bg.tile([128, NLOC], BF16, tag="xT", bufs=2)
            transpose_in(xT, x_in)

            def dense_group(gidx, out_T, rhs1_T, w1, rhs2_T, w2, bias_col,
                            residual_T):
                lo, hi = gidx * 512, min((gidx + 1) * 512, NLOC)
                n = hi - lo
                ps_h = ps.tile([128, 512], F32, space="PSUM", tag="dense")
                nc.tensor.matmul(out=ps_h[:, :n], lhsT=w1, rhs=rhs1_T[:, lo:hi],
                                 start=True, stop=(rhs2_T is None))
                if rhs2_T is not None:
                    nc.tensor.matmul(out=ps_h[:, :n], lhsT=w2,
                                     rhs=rhs2_T[:, lo:hi], start=False,
                                     stop=True)
                if residual_T is None:
                    nc.scalar.activation(out=out_T[:, lo:hi], in_=ps_h[:, :n],
                                         func=RELU, bias=bias_col, scale=1.0)
                else:
                    h_sb = wk.tile([128, 512], BF16, tag="hsb")
                    nc.scalar.activation(out=h_sb[:, :n], in_=ps_h[:, :n],
                                         func=RELU, bias=bias_col, scale=1.0)
                    nc.vector.tensor_add(out=out_T[:, lo:hi],
                                         in0=residual_T[:, lo:hi],
                                         in1=h_sb[:, :n])
                return lo, hi

            def publish_group(cc_dram, lo, hi, src_T):
                for i in range(lo // 128, -(-hi // 128)):
                    transpose_tile(src_T, i)
                publish_rows(cc_dram, lo, hi)

            def seg_allgather(cc_dram, xtab_dram, s):
                ln = L[s + 1] - L[s]
                if "ag" in ablate:
                    # stand-in with the same local DMA-engine cost: 8-fold copy
                    for r in range(C):
                        nc.sync.dma_start(
                            out=xtab_dram[C * L[s] + r * ln:
                                          C * L[s] + (r + 1) * ln, :],
                            in_=cc_dram[L[s]:L[s + 1], :])
                else:
                    # Shared tiles allow only one writer, so gather segments
                    # into per-segment tiles and copy into the unified table.
                    xseg = dr.tile([C * ln, D], BF16, tag=f"xseg{s}",
                                   addr_space="Shared", bufs=2)
                    nc.gpsimd.collective_compute(
                        "AllGather", mybir.AluOpType.bypass, replica_groups=rg,
                        ins=[cc_dram[L[s]:L[s + 1], :].opt()],
                        outs=[xseg[:].opt()],
                    )
                    nc.sync.dma_start(out=xtab_dram[C * L[s]:C * L[s + 1], :],
                                      in_=xseg[:])

            # project x -> xT1, publish + segment AllGathers -> xtab
            xT1 = bg.tile([128, NLOC], BF16, tag="xT", bufs=2)
            cc_cur = dr.tile([n_tp * 128, D], BF16, tag="cc_in", bufs=2)
            xtab = dr.tile([cfg.N, D], BF16, tag="xtab", bufs=2)
            seg_done = 0
            for gidx in range(n_dense):
                lo, hi = dense_group(gidx, xT1, xT, W(0), None, None, B(0),
                                     None)
                publish_group(cc_cur, lo, hi, xT1)
                while seg_done < NSEG and gidx == seg_trigger[seg_done]:
                    seg_allgather(cc_cur, xtab, seg_done)
                    seg_done += 1
            cur_xT = xT1

            # ---------- 4 SAGE layers ----------
            aggT = bg.tile([128, NLOC], BF16, tag="aggT", bufs=1)
            for layer in range(4):
                new_xT = bg.tile([128, NLOC], BF16, tag="xT", bufs=2)
                if layer < 3:
                    cc_next = dr.tile([n_tp * 128, D], BF16, tag="cc_in",
                                      bufs=2)
                    xtab_next = dr.tile([cfg.N, D], BF16, tag="xtab",
                                        bufs=2)
                g_done = 0
                seg_done = 0
                xtab_pairs = xtab[:].rearrange("(r two) d -> r (two d)", two=2)

                def finish_groups(w, layer=layer, new_xT=new_xT,
                                  cc=cc_next if layer < 3 else None,
                                  xt=xtab_next if layer < 3 else None,
                                  cur=cur_xT):
                    nonlocal g_done, seg_done
                    while g_done < n_dense and (
                            (w + 1) * WIN >= (g_done + 1) * 512
                            or w == NWIN - 1):
                        lo, hi = dense_group(
                            g_done, new_xT, aggT, W(1 + layer), cur,
                            W(5 + layer), B(1 + layer),
                            cur if layer < 3 else None)
                        if layer < 3:
                            publish_group(cc, lo, hi, new_xT)
                            while (seg_done < NSEG
                                   and g_done == seg_trigger[seg_done]):
                                seg_allgather(cc, xt, seg_done)
                                seg_done += 1
                        else:
                            # epilogue: fp32 rows straight to y_out
                            for i in range(lo // 128, -(-hi // 128)):
                                rlo = i * 128
                                rhi = min((i + 1) * 128, NLOC)
                                p = rhi - rlo
                                ps_t = ps.tile([128, 128], BF16, space="PSUM",
                                               tag="tp")
                                nc.tensor.transpose(
                                    out=ps_t[:p, :], in_=new_xT[:, rlo:rhi],
                                    identity=ident[:, :])
                                yr = wk.tile([128, 128], F32, tag="yr")
                                nc.vector.tensor_copy(out=yr[:p, :],
                                                      in_=ps_t[:p, :])
                                nc.sync.dma_start(out=y_out[rlo:rhi, :],
                                                  in_=yr[:p, :])
                        g_done += 1

                RC = cfg.batch_chunks
                g_tiles = {}
                next_run = [0]

                def ensure_runs(upto_slot):
                    while next_run[0] * RC < min(upto_slot, T):
                        r = next_run[0]
                        t0, t1 = r * RC, min((r + 1) * RC, T)
                        nb = t1 - t0
                        g = wk.tile([128, RC * 256], BF16, tag="g", bufs=4)
                        g_tiles[r] = g
                        if "gather" in ablate:
                            nc.vector.memset(g[:, :nb * 256], 0.0)
                        else:
                            nc.gpsimd.dma_gather(
                                g[:, :nb * 256].rearrange(
                                    "p (c e) -> p c e", e=256),
                                xtab_pairs, idx_sb[:, 8 * t0:8 * t1],
                                nb * CH, nb * CH, 256, queue_num=r % 4)
                        g_tiles.pop(r - 4, None)
                        next_run[0] += 1

                for w in range(NWIN):
                    nw = nch_w[w]
                    tw = int(slot0[w])
                    lo = w * WIN
                    hi = min(lo + WIN, NLOC)
                    ensure_runs(tw + nw + RC)   # prefetch one run ahead
                    onehot = wk.tile([128, 2 * maxn * WIN], BF16,
                                     tag="onehot", bufs=4)
                    nc.vector.tensor_tensor(
                        out=onehot[:, :2 * nw * WIN].rearrange(
                            "p (c x) -> p c x", x=WIN),
                        in0=dsteo_sb[:, 2 * tw:2 * (tw + nw)].to_broadcast(
                            [128, 2 * nw, WIN]),
                        in1=iota_sb[:, :2 * nw * WIN].rearrange(
                            "p (c x) -> p c x", x=WIN),
                        op=mybir.AluOpType.is_equal,
                    )
                    ps_w = ps.tile([128, WIN], F32, space="PSUM", tag="seg")
                    for k in range(nw):
                        t = tw + k
                        g = g_tiles[t // RC]
                        cb = t % RC
                        nc.tensor.matmul(
                            out=ps_w[:],
                            lhsT=g[:, cb * 256:cb * 256 + 128],
                            rhs=onehot[:, 2 * k * WIN:(2 * k + 1) * WIN],
                            start=(k == 0), stop=False,
                        )
                        nc.tensor.matmul(
                            out=ps_w[:],
                            lhsT=g[:, cb * 256 + 128:cb * 256 + 256],
                            rhs=onehot[:,
                                       (2 * k + 1) * WIN:(2 * k + 2) * WIN],
                            start=False, stop=(k == nw - 1),
                        )
                    nc.vector.tensor_mul(out=aggT[:, lo:hi],
                                         in0=ps_w[:, :hi - lo],
                                         in1=invdeg_bc[:, lo:hi])
                    finish_groups(w)

                finish_groups(NWIN - 1)
                if layer < 3:
                    xtab = xtab_next
                cur_xT = new_xT

    nc.compile()
    return nc


@dataclass
class Built:
    cfg: Cfg
    nc: object
    sched: dict


_built_cache: dict = {}


def build(cfg: Cfg, edge_index: np.ndarray) -> Built:
    sched = preprocess(cfg, edge_index)
    key = (cfg.N, cfg.E, cfg.C, tuple(sched["nch_w"]))
    if key not in _built_cache:
        nc = build_program(cfg, sched["nch_w"], sched["batches"], sched["T"],
                           sched["maxn"])
        _built_cache[key] = nc
    return Built(cfg, _built_cache[key], sched)


def make_in_maps(cfg: Cfg, built: Built, x, Wp, bp, Wl, bl, Wr,
                 bn_gamma, bn_beta, bn_mean, bn_var):
    sched = built.sched
    Wl_f, Wr_f, c_f = fold_weights(Wp, bp, Wl, bl, Wr,
                                   bn_gamma, bn_beta, bn_mean, bn_var)
    weights = np.stack([Wp] + list(Wl_f) + list(Wr_f)).astype(BF)
    biases = np.stack([bp] + list(c_f)
                      + [np.zeros_like(bp)] * 4).astype(np.float32)
    x_perm = np.ascontiguousarray(x[sched["sigma"]]).astype(BF)
    invdeg_perm = sched["inv_deg"]  # already in permuted (new-id) order
    dst_eo_bf = sched["dst_eo"].astype(BF)
    iota_bf = sched["iota2"].astype(BF)
    in_maps = []
    for c in range(cfg.C):
        lo = c * cfg.NLOC
        in_maps.append({
            "x_slice": np.ascontiguousarray(x_perm[lo:lo + cfg.NLOC]),
            "idx16": np.ascontiguousarray(sched["idx16"][c]),
            "dst_eo": np.ascontiguousarray(dst_eo_bf[c]),
            "iota2": iota_bf,
            "inv_deg_sl": np.ascontiguousarray(
                invdeg_perm[None, lo:lo + cfg.NLOC]),
            "weights": weights,
            "biases": biases,
        })
    return in_maps


def kernel(x, edge_index, Wp, bp, Wl, bl, Wr, bn_gamma, bn_beta, bn_mean,
           bn_var) -> np.ndarray:
    from concourse.bass_utils import run_bass_kernel_spmd

    cfg = Cfg()
    x = np.asarray(x, np.float32)
    edge_index = np.asarray(edge_index)
    built = build(cfg, edge_index)
    in_maps = make_in_maps(cfg, built, x, np.asarray(Wp), np.asarray(bp),
                           np.asarray(Wl), np.asarray(bl), np.asarray(Wr),
                           np.asarray(bn_gamma), np.asarray(bn_beta),
                           np.asarray(bn_mean), np.asarray(bn_var))
    res = run_bass_kernel_spmd(built.nc, in_maps, core_ids=list(range(cfg.C)))
    out = np.concatenate([res.results[c]["y"] for c in range(cfg.C)], axis=0)
    return out[built.sched["pi"]].astype(np.float32)
